# revision 25
# baseline (speedup 1.0000x reference)
"""Multi-head attention (B=4, S=2048, E=1024, H=16, D=64) on 8 TRN2 NeuronCores.

Sharding: core c handles batch b = c//2 and heads [8*(c%2), 8*(c%2)+8) —
data parallel over batch, tensor parallel over heads. No collectives:
each core computes its own output slice, gathered on host.

v3: all-bf16 dataflow (inputs cast on host; f32 PSUM accumulation).
Projections are split into 8-matmul groups; a prelude computes what the
first attention head needs, the rest are interleaved into the attention
loop where the Activation engine (exp) is the bottleneck and the PE has
slack. PSUM budget: scores ring 2x[128,2,512] (4 banks) + context
accumulator [65,1024] (2 banks) + projection accumulator ring (2 banks).

Per-core algorithm:
  qT[f, s] / kT[f, s] = W^T X^T  (feature-major; head 2m on partitions
  0-63 of chunk m, head 2m+1 on 64-127)
  vaug[sk, h, t, 0:64] = V; vaug[.., 64] = 1
  per head h, sq-half u, sk-block t:
    st[sk, j]   = kT_h[:, t]^T @ qT_h[:, j],  j = 2u, 2u+1   (K=64, N=512)
    ex          = exp(st / 8)                 (one ACT per t, bf16 out)
    ctx[0:65, j] += vaug_h[t]^T @ ex[:, j]    (K=128, N=512)
  out[h] = ctx  ([65, 2048] bf16; row 64 = softmax denominator, divided
  out on host)
"""

import numpy as np
import ml_dtypes
from contextlib import ExitStack

import concourse.bass as bass
import concourse.tile as tile
from concourse import bacc
from concourse import mybir
from concourse.bass_utils import run_bass_kernel_spmd

F32 = mybir.dt.float32
BF16 = mybir.dt.bfloat16
EXP = mybir.ActivationFunctionType.Exp
NP_BF16 = ml_dtypes.bfloat16

B, S, E = 4, 2048, 1024
H, D = 16, 64
HPC = 8            # heads per core
FPC = HPC * D      # 512 output features per core
N_CORES = 8
KC = E // 128      # contraction chunks
NT = S // 128      # sk blocks
SCALE = 0.125      # 1/sqrt(64)


def build_bass():
    nc = bacc.Bacc()
    xq = nc.declare_dram_parameter("xq_t", [E, S], BF16, isOutput=False)
    xk = nc.declare_dram_parameter("xk_t", [E, S], BF16, isOutput=False)
    xv = nc.declare_dram_parameter("xv_t", [E, S], BF16, isOutput=False)
    wq = nc.declare_dram_parameter("wq", [E, FPC], BF16, isOutput=False)
    wk = nc.declare_dram_parameter("wk", [E, FPC], BF16, isOutput=False)
    wv = nc.declare_dram_parameter("wv", [E, FPC], BF16, isOutput=False)
    out = nc.declare_dram_parameter("out", [HPC, D + 1, S], BF16, isOutput=True)

    with tile.TileContext(nc) as tc, ExitStack() as ctx:
        sb = ctx.enter_context(tc.tile_pool(name="sb", bufs=1))
        exp = ctx.enter_context(tc.tile_pool(name="exp", bufs=6))
        csb = ctx.enter_context(tc.tile_pool(name="csb", bufs=3))
        pproj = ctx.enter_context(tc.tile_pool(name="pproj", bufs=2,
                                               space="PSUM"))

        # --- weights + activations, resident in SBUF (bf16) ---
        w_sb = {}
        for name, w in (("wq", wq), ("wk", wk), ("wv", wv)):
            t = sb.tile([128, KC, FPC], BF16, name=f"{name}_sb", tag=f"{name}_sb")
            nc.sync.dma_start(out=t, in_=w.rearrange("(kc p) f -> p kc f", p=128))
            w_sb[name] = t
        x_sb = {}
        for name, x in (("xk", xk), ("xq", xq), ("xv", xv)):
            x_sb[name] = sb.tile([128, KC, S], BF16, name=f"{name}_sb",
                                 tag=f"{name}_sb")
        # quarter-column DMAs, issued in first-consumer order, so the first
        # projection groups (each needs one 512-column span) start early
        x_dram = {"xk": xk, "xq": xq, "xv": xv}
        for name, uu in (("xk", 0), ("xk", 1), ("xk", 2), ("xk", 3),
                         ("xq", 0), ("xq", 1), ("xv", 0), ("xv", 1),
                         ("xq", 2), ("xq", 3), ("xv", 2), ("xv", 3)):
            nc.sync.dma_start(
                out=x_sb[name][:, :, uu * 512:(uu + 1) * 512],
                in_=x_dram[name][:, uu * 512:(uu + 1) * 512].rearrange(
                    "(kc p) s -> p kc s", p=128))

        # --- persistent projection outputs ---
        qT = sb.tile([128, 4, S], BF16, name="qT", tag="qT")   # [f%128, f//128, s]
        kT = sb.tile([128, 4, S], BF16, name="kT", tag="kT")
        vaug = sb.tile([128, HPC, NT, D + 1], BF16, name="vaug", tag="vaug")
        nc.vector.memset(vaug[:, :, :, D:D + 1], 1.0)

        # --- projection group emitters (8 accumulating matmuls + 1 copy) ---
        def qk_group(name, dst, m, u):
            def emit():
                acc = pproj.tile([128, 512], F32, name=f"p_{name}_{m}_{u}",
                                 tag="proj")
                for kc in range(KC):
                    nc.tensor.matmul(
                        acc,
                        lhsT=w_sb[f"w{name}"][:, kc, m * 128:(m + 1) * 128],
                        rhs=x_sb[f"x{name}"][:, kc, u * 512:(u + 1) * 512],
                        start=(kc == 0), stop=(kc == KC - 1),
                    )
                nc.vector.tensor_copy(
                    out=dst[:, m, u * 512:(u + 1) * 512], in_=acc)
            return emit

        def qk_units(name, dst, m, u, span):
            # qk_group split into pop-units of `span` matmuls (last unit
            # adds the copy), so interleaved pops stay under the ACT-bound
            # iteration budget instead of stalling ACT ~1.6us per 8-MM lump
            state = {}

            def unit(i):
                def emit():
                    if i == 0:
                        state["acc"] = pproj.tile(
                            [128, 512], F32, name=f"p_{name}_{m}_{u}",
                            tag="proj")
                    acc = state["acc"]
                    for kc in range(i * span, (i + 1) * span):
                        nc.tensor.matmul(
                            acc,
                            lhsT=w_sb[f"w{name}"][:, kc, m * 128:(m + 1) * 128],
                            rhs=x_sb[f"x{name}"][:, kc, u * 512:(u + 1) * 512],
                            start=(kc == 0), stop=(kc == KC - 1),
                        )
                    if i == KC // span - 1:
                        nc.vector.tensor_copy(
                            out=dst[:, m, u * 512:(u + 1) * 512], in_=acc)
                return emit
            return [unit(i) for i in range(KC // span)]

        def v_group(sc):
            def emit():
                acc = pproj.tile([128, FPC], F32, name=f"p_v_{sc}", tag="proj")
                for kc in range(KC):
                    nc.tensor.matmul(
                        acc,
                        lhsT=x_sb["xv"][:, kc, sc * 128:(sc + 1) * 128],
                        rhs=w_sb["wv"][:, kc, :],
                        start=(kc == 0), stop=(kc == KC - 1),
                    )
                nc.vector.tensor_copy(
                    out=vaug[:, :, sc, 0:D],
                    in_=acc.rearrange("p (h d) -> p h d", h=HPC))
            return emit

        # Prelude: the minimum head 0's first iterations need.
        for uu in range(4):
            qk_group("k", kT, 0, uu)()
        qk_group("q", qT, 0, 0)()
        qk_group("q", qT, 0, 1)()
        v_group(0)()
        v_group(1)()

        # Prelude also covers all of q m0 (head 0 uses quarters 2/3 from
        # iteration 16, before any deferred qk work could deliver them).
        qk_group("q", qT, 0, 2)()
        qk_group("q", qT, 0, 3)()

        # Deferred, popped one per attention iteration:
        # - v blocks sc=2..15 as whole groups, iters 0..13 (ctx at iteration
        #   t needs block t, so v must stay a full group ahead);
        # - m=1 k/q as 2-matmul units, iters 14..45 (deadline: iter 64);
        # - m=2,3 as 1-matmul units (215ns, fits the ~385ns/iter PE slack
        #   under the ACT-bound pace), iters 46..173 (deadlines 128/192).
        deferred = [v_group(sc) for sc in range(2, NT)]
        for name, dst in (("k", kT), ("q", qT)):
            for uu in range(4):
                deferred.extend(qk_units(name, dst, 1, uu, span=2))
        for mm in (2, 3):
            for name, dst in (("k", kT), ("q", qT)):
                for uu in range(4):
                    deferred.extend(qk_units(name, dst, mm, uu, span=1))

        # --- attention ---
        stp = ctx.enter_context(tc.tile_pool(name="stp", bufs=2, space="PSUM"))
        ctxp = ctx.enter_context(tc.tile_pool(name="ctxp", bufs=1, space="PSUM"))
        it = 0   # global t-iteration index, for deferred-group scheduling
        for h in range(HPC):
            po = (h % 2) * 64   # partition offset of head h inside its chunk
            m = h // 2
            for u in range(2):
                cacc = ctxp.tile([D + 1, S // 2], F32, name=f"ctx_{h}_{u}",
                                 tag="ctx")
                for t in range(NT):
                    st = stp.tile([128, 2, 512], F32, name=f"st_{h}_{u}_{t}",
                                  tag="st")
                    for v in range(2):
                        j = 2 * u + v
                        nc.tensor.matmul(
                            st[:, v, :],
                            lhsT=kT[po:po + 64, m, t * 128:(t + 1) * 128],
                            rhs=qT[po:po + 64, m, j * 512:(j + 1) * 512],
                            start=True, stop=True,
                        )
                    ex = exp.tile([128, 2, 512], BF16, name=f"ex_{h}_{u}_{t}",
                                  tag="ex")
                    nc.scalar.activation(ex, st, EXP, scale=SCALE)
                    for v in range(2):
                        nc.tensor.matmul(
                            cacc[:, v * 512:(v + 1) * 512],
                            lhsT=vaug[:, h, t, :],
                            rhs=ex[:, v, :],
                            start=(t == 0), stop=(t == NT - 1),
                        )
                    # interleave deferred projection work into PE slack
                    if deferred:
                        deferred.pop(0)()
                    it += 1
                cs = csb.tile([D + 1, S // 2], BF16, name=f"csb_{h}_{u}",
                              tag="csb")
                nc.vector.tensor_copy(out=cs, in_=cacc)
                nc.sync.dma_start(
                    out=out[h, :, u * 1024:(u + 1) * 1024], in_=cs)
        while deferred:
            deferred.pop(0)()

    nc.compile()
    nc.freeze()
    return nc


_NC_CACHE = None


def _get_nc():
    global _NC_CACHE
    if _NC_CACHE is None:
        _NC_CACHE = build_bass()
    return _NC_CACHE


def make_in_maps(queries, keys, values, Wq, Wk, Wv):
    # Host-side shard prep: transpose + cast to bf16 once per batch,
    # slice W by head group.
    xq_t = [np.ascontiguousarray(queries[b].T).astype(NP_BF16) for b in range(B)]
    xk_t = [np.ascontiguousarray(keys[b].T).astype(NP_BF16) for b in range(B)]
    xv_t = [np.ascontiguousarray(values[b].T).astype(NP_BF16) for b in range(B)]
    w_half = [
        (np.ascontiguousarray(Wq[:, g * FPC:(g + 1) * FPC]).astype(NP_BF16),
         np.ascontiguousarray(Wk[:, g * FPC:(g + 1) * FPC]).astype(NP_BF16),
         np.ascontiguousarray(Wv[:, g * FPC:(g + 1) * FPC]).astype(NP_BF16))
        for g in range(2)
    ]

    in_maps = []
    for c in range(N_CORES):
        b, g = c // 2, c % 2
        in_maps.append({
            "xq_t": xq_t[b], "xk_t": xk_t[b], "xv_t": xv_t[b],
            "wq": w_half[g][0], "wk": w_half[g][1], "wv": w_half[g][2],
        })
    return in_maps


def kernel(queries, keys, values, Wq, Wk, Wv, **_):
    queries = np.asarray(queries, dtype=np.float32)
    keys = np.asarray(keys, dtype=np.float32)
    values = np.asarray(values, dtype=np.float32)
    Wq = np.asarray(Wq, dtype=np.float32)
    Wk = np.asarray(Wk, dtype=np.float32)
    Wv = np.asarray(Wv, dtype=np.float32)

    in_maps = make_in_maps(queries, keys, values, Wq, Wk, Wv)
    nc = _get_nc()
    res = run_bass_kernel_spmd(nc, in_maps, list(range(N_CORES)))

    full = np.empty((B, S, H * D), dtype=np.float32)
    for c in range(N_CORES):
        b, g = c // 2, c % 2
        o = res.results[c]["out"].astype(np.float32)   # [HPC, D+1, S]
        ctx = o[:, :D, :] / o[:, D:D + 1, :]           # [HPC, D, S]
        dst = full[b].reshape(S, H, D)
        dst[:, g * HPC:(g + 1) * HPC, :] = ctx.transpose(2, 0, 1)
    return full


# revision 31
# speedup vs baseline: 4.1449x; 4.1449x over previous
"""Multi-head attention (B=4, S=2048, E=1024, H=16, D=64) on 8 TRN2 NeuronCores.

Sharding: core c handles batch b = c//2 and heads [8*(c%2), 8*(c%2)+8) —
data parallel over batch, tensor parallel over heads. No collectives:
each core computes its own output slice, gathered on host.

v3: all-bf16 dataflow (inputs cast on host; f32 PSUM accumulation).
Projections are split into 8-matmul groups; a prelude computes what the
first attention head needs, the rest are interleaved into the attention
loop where the Activation engine (exp) is the bottleneck and the PE has
slack. PSUM budget: scores ring 2x[128,2,512] (4 banks) + context
accumulator [65,1024] (2 banks) + projection accumulator ring (2 banks).

Per-core algorithm:
  qT[f, s] / kT[f, s] = W^T X^T  (feature-major; head 2m on partitions
  0-63 of chunk m, head 2m+1 on 64-127)
  vaug[sk, h, t, 0:64] = V; vaug[.., 64] = 1
  per head h, sq-half u, sk-block t:
    st[sk, j]   = kT_h[:, t]^T @ qT_h[:, j],  j = 2u, 2u+1   (K=64, N=512)
    ex          = exp(st / 8)                 (one ACT per t, bf16 out)
    ctx[0:65, j] += vaug_h[t]^T @ ex[:, j]    (K=128, N=512)
  out[h] = ctx  ([65, 2048] bf16; row 64 = softmax denominator, divided
  out on host)
"""

import numpy as np
import ml_dtypes
from contextlib import ExitStack

import concourse.bass as bass
import concourse.tile as tile
from concourse import bacc
from concourse import mybir
from concourse.bass_utils import run_bass_kernel_spmd

F32 = mybir.dt.float32
BF16 = mybir.dt.bfloat16
EXP = mybir.ActivationFunctionType.Exp
NP_BF16 = ml_dtypes.bfloat16

B, S, E = 4, 2048, 1024
H, D = 16, 64
HPC = 8            # heads per core
FPC = HPC * D      # 512 output features per core
N_CORES = 8
KC = E // 128      # contraction chunks
NT = S // 128      # sk blocks
SCALE = 0.125      # 1/sqrt(64)


def build_bass():
    nc = bacc.Bacc()
    xq = nc.declare_dram_parameter("xq_t", [E, S], BF16, isOutput=False)
    xk = nc.declare_dram_parameter("xk_t", [E, S], BF16, isOutput=False)
    xv = nc.declare_dram_parameter("xv_t", [E, S], BF16, isOutput=False)
    wq = nc.declare_dram_parameter("wq", [E, FPC], BF16, isOutput=False)
    wk = nc.declare_dram_parameter("wk", [E, FPC], BF16, isOutput=False)
    wv = nc.declare_dram_parameter("wv", [E, FPC], BF16, isOutput=False)
    out = nc.declare_dram_parameter("out", [HPC, D + 1, S], BF16, isOutput=True)

    with tile.TileContext(nc) as tc, ExitStack() as ctx:
        sb = ctx.enter_context(tc.tile_pool(name="sb", bufs=1))
        exp = ctx.enter_context(tc.tile_pool(name="exp", bufs=6))
        csb = ctx.enter_context(tc.tile_pool(name="csb", bufs=3))
        pproj = ctx.enter_context(tc.tile_pool(name="pproj", bufs=2,
                                               space="PSUM"))

        # Dummy exp on a scratch tile: triggers the ~2.7us exp-table load
        # (PSEUDO_LOAD_ACT_FUNC_SET) while the input DMAs stream, instead
        # of on the critical path at the first real exp.
        warm = sb.tile([128, 1], F32, name="act_warm", tag="act_warm")
        nc.vector.memset(warm, 0.0)
        nc.scalar.activation(warm, warm, EXP)

        # --- weights + activations, resident in SBUF (bf16) ---
        w_sb = {}
        for name, w in (("wq", wq), ("wk", wk), ("wv", wv)):
            t = sb.tile([128, KC, FPC], BF16, name=f"{name}_sb", tag=f"{name}_sb")
            nc.sync.dma_start(out=t, in_=w.rearrange("(kc p) f -> p kc f", p=128))
            w_sb[name] = t
        x_sb = {}
        for name, x in (("xk", xk), ("xq", xq), ("xv", xv)):
            x_sb[name] = sb.tile([128, KC, S], BF16, name=f"{name}_sb",
                                 tag=f"{name}_sb")
        # quarter-column DMAs, issued in first-consumer order, so the first
        # projection groups (each needs one 512-column span) start early
        x_dram = {"xk": xk, "xq": xq, "xv": xv}
        for name, uu in (("xk", 0), ("xk", 1), ("xk", 2), ("xk", 3),
                         ("xq", 0), ("xq", 1), ("xv", 0), ("xv", 1),
                         ("xq", 2), ("xq", 3), ("xv", 2), ("xv", 3)):
            nc.sync.dma_start(
                out=x_sb[name][:, :, uu * 512:(uu + 1) * 512],
                in_=x_dram[name][:, uu * 512:(uu + 1) * 512].rearrange(
                    "(kc p) s -> p kc s", p=128))

        # --- persistent projection outputs ---
        qT = sb.tile([128, 4, S], BF16, name="qT", tag="qT")   # [f%128, f//128, s]
        kT = sb.tile([128, 4, S], BF16, name="kT", tag="kT")
        vaug = sb.tile([128, HPC, NT, D + 1], BF16, name="vaug", tag="vaug")
        nc.vector.memset(vaug[:, :, :, D:D + 1], 1.0)

        # --- projection group emitters (8 accumulating matmuls + 1 copy) ---
        def qk_group(name, dst, m, u):
            def emit():
                acc = pproj.tile([128, 512], F32, name=f"p_{name}_{m}_{u}",
                                 tag="proj")
                for kc in range(KC):
                    nc.tensor.matmul(
                        acc,
                        lhsT=w_sb[f"w{name}"][:, kc, m * 128:(m + 1) * 128],
                        rhs=x_sb[f"x{name}"][:, kc, u * 512:(u + 1) * 512],
                        start=(kc == 0), stop=(kc == KC - 1),
                    )
                nc.vector.tensor_copy(
                    out=dst[:, m, u * 512:(u + 1) * 512], in_=acc)
            return emit

        def qk_units(name, dst, m, u, span):
            # qk_group split into pop-units of `span` matmuls (last unit
            # adds the copy), so interleaved pops stay under the ACT-bound
            # iteration budget instead of stalling ACT ~1.6us per 8-MM lump
            state = {}

            def unit(i):
                def emit():
                    if i == 0:
                        state["acc"] = pproj.tile(
                            [128, 512], F32, name=f"p_{name}_{m}_{u}",
                            tag="proj")
                    acc = state["acc"]
                    for kc in range(i * span, (i + 1) * span):
                        nc.tensor.matmul(
                            acc,
                            lhsT=w_sb[f"w{name}"][:, kc, m * 128:(m + 1) * 128],
                            rhs=x_sb[f"x{name}"][:, kc, u * 512:(u + 1) * 512],
                            start=(kc == 0), stop=(kc == KC - 1),
                        )
                    if i == KC // span - 1:
                        nc.vector.tensor_copy(
                            out=dst[:, m, u * 512:(u + 1) * 512], in_=acc)
                return emit
            return [unit(i) for i in range(KC // span)]

        def v_group(sc):
            def emit():
                acc = pproj.tile([128, FPC], F32, name=f"p_v_{sc}", tag="proj")
                for kc in range(KC):
                    nc.tensor.matmul(
                        acc,
                        lhsT=x_sb["xv"][:, kc, sc * 128:(sc + 1) * 128],
                        rhs=w_sb["wv"][:, kc, :],
                        start=(kc == 0), stop=(kc == KC - 1),
                    )
                nc.vector.tensor_copy(
                    out=vaug[:, :, sc, 0:D],
                    in_=acc.rearrange("p (h d) -> p h d", h=HPC))
            return emit

        # Prelude: the minimum head 0's first iterations need.
        for uu in range(4):
            qk_group("k", kT, 0, uu)()
        qk_group("q", qT, 0, 0)()
        qk_group("q", qT, 0, 1)()
        v_group(0)()
        v_group(1)()

        # Prelude also covers all of q m0 (head 0 uses quarters 2/3 from
        # iteration 16, before any deferred qk work could deliver them).
        qk_group("q", qT, 0, 2)()
        qk_group("q", qT, 0, 3)()

        # Deferred, popped one per attention iteration:
        # - v blocks sc=2..15 as whole groups, iters 0..13 (ctx at iteration
        #   t needs block t, so v must stay a full group ahead);
        # - m=1 k/q as 2-matmul units, iters 14..45 (deadline: iter 64);
        # - m=2,3 as 1-matmul units (215ns, fits the ~385ns/iter PE slack
        #   under the ACT-bound pace), iters 46..173 (deadlines 128/192).
        deferred = [v_group(sc) for sc in range(2, NT)]
        for name, dst in (("k", kT), ("q", qT)):
            for uu in range(4):
                deferred.extend(qk_units(name, dst, 1, uu, span=2))
        for mm in (2, 3):
            for name, dst in (("k", kT), ("q", qT)):
                for uu in range(4):
                    deferred.extend(qk_units(name, dst, mm, uu, span=1))

        # --- attention ---
        stp = ctx.enter_context(tc.tile_pool(name="stp", bufs=2, space="PSUM"))
        ctxp = ctx.enter_context(tc.tile_pool(name="ctxp", bufs=1, space="PSUM"))
        it = 0   # global t-iteration index, for deferred-group scheduling
        for h in range(HPC):
            po = (h % 2) * 64   # partition offset of head h inside its chunk
            m = h // 2
            for u in range(2):
                cacc = ctxp.tile([D + 1, S // 2], F32, name=f"ctx_{h}_{u}",
                                 tag="ctx")
                for t in range(NT):
                    st = stp.tile([128, 2, 512], F32, name=f"st_{h}_{u}_{t}",
                                  tag="st")
                    for v in range(2):
                        j = 2 * u + v
                        nc.tensor.matmul(
                            st[:, v, :],
                            lhsT=kT[po:po + 64, m, t * 128:(t + 1) * 128],
                            rhs=qT[po:po + 64, m, j * 512:(j + 1) * 512],
                            start=True, stop=True,
                        )
                    ex = exp.tile([128, 2, 512], BF16, name=f"ex_{h}_{u}_{t}",
                                  tag="ex")
                    nc.scalar.activation(ex, st, EXP, scale=SCALE)
                    for v in range(2):
                        nc.tensor.matmul(
                            cacc[:, v * 512:(v + 1) * 512],
                            lhsT=vaug[:, h, t, :],
                            rhs=ex[:, v, :],
                            start=(t == 0), stop=(t == NT - 1),
                        )
                    # interleave deferred projection work into PE slack
                    if deferred:
                        deferred.pop(0)()
                    it += 1
                cs = csb.tile([D + 1, S // 2], BF16, name=f"csb_{h}_{u}",
                              tag="csb")
                nc.vector.tensor_copy(out=cs, in_=cacc)
                nc.sync.dma_start(
                    out=out[h, :, u * 1024:(u + 1) * 1024], in_=cs)
        while deferred:
            deferred.pop(0)()

    nc.compile()
    nc.freeze()
    return nc


_NC_CACHE = None


def _get_nc():
    global _NC_CACHE
    if _NC_CACHE is None:
        _NC_CACHE = build_bass()
    return _NC_CACHE


def make_in_maps(queries, keys, values, Wq, Wk, Wv):
    # Host-side shard prep: transpose + cast to bf16 once per batch,
    # slice W by head group.
    xq_t = [np.ascontiguousarray(queries[b].T).astype(NP_BF16) for b in range(B)]
    xk_t = [np.ascontiguousarray(keys[b].T).astype(NP_BF16) for b in range(B)]
    xv_t = [np.ascontiguousarray(values[b].T).astype(NP_BF16) for b in range(B)]
    w_half = [
        (np.ascontiguousarray(Wq[:, g * FPC:(g + 1) * FPC]).astype(NP_BF16),
         np.ascontiguousarray(Wk[:, g * FPC:(g + 1) * FPC]).astype(NP_BF16),
         np.ascontiguousarray(Wv[:, g * FPC:(g + 1) * FPC]).astype(NP_BF16))
        for g in range(2)
    ]

    in_maps = []
    for c in range(N_CORES):
        b, g = c // 2, c % 2
        in_maps.append({
            "xq_t": xq_t[b], "xk_t": xk_t[b], "xv_t": xv_t[b],
            "wq": w_half[g][0], "wk": w_half[g][1], "wv": w_half[g][2],
        })
    return in_maps


def kernel(queries, keys, values, Wq, Wk, Wv, **_):
    queries = np.asarray(queries, dtype=np.float32)
    keys = np.asarray(keys, dtype=np.float32)
    values = np.asarray(values, dtype=np.float32)
    Wq = np.asarray(Wq, dtype=np.float32)
    Wk = np.asarray(Wk, dtype=np.float32)
    Wv = np.asarray(Wv, dtype=np.float32)

    in_maps = make_in_maps(queries, keys, values, Wq, Wk, Wv)
    nc = _get_nc()
    res = run_bass_kernel_spmd(nc, in_maps, list(range(N_CORES)))

    full = np.empty((B, S, H * D), dtype=np.float32)
    for c in range(N_CORES):
        b, g = c // 2, c % 2
        o = res.results[c]["out"].astype(np.float32)   # [HPC, D+1, S]
        ctx = o[:, :D, :] / o[:, D:D + 1, :]           # [HPC, D, S]
        dst = full[b].reshape(S, H, D)
        dst[:, g * HPC:(g + 1) * HPC, :] = ctx.transpose(2, 0, 1)
    return full
